# revision 65
# baseline (speedup 1.0000x reference)
"""Trainium2 Bass kernel: fused cross-head attention block (data parallel, 8 cores).

Problem (hardcoded shapes): x_c, x_t [8, 256, 128, 128] f32; Wq/Wk/Wv/Wo
[256, 256]; biases [256]; out [8, 256, 128, 128] f32.

Math per sample (C=256, nh=8, hd=32, N=H*W=16384 tokens):
  x = x_c + x_t;  q/k/v = per-token linear projections of x
  per token: dot[h,g] = q_h . k_g * hd^-0.5   (8x8 gram ACROSS heads)
             attn = softmax_g(dot);  o_h = sum_g attn[h,g] v_g
  out = Wo @ o (+ bo)

Sharding: pure data parallel - one sample per NeuronCore, weights replicated.

Per core: 32 groups of 4x128-token tiles, software-pipelined in 6 phases
(lags in groups) so every in-order engine queue sees work dependency-ready:
  load(g):   DMA xc/xt; fp16 casts on ACT (the x-sum rides PE's PSUM
             accumulation over both sources in the qkv matmuls).
  proj(g):   PE qkv matmuls (384-wide PSUM half-tiles, dual-source + c-half
             accumulated); ACT evacuation; DVE per-tile gram multiply
             (broadcast APs) + ONE c-tree level (32 -> 16).
  dots(g-1): the remaining c-reduction rides PE: 16 parity-accumulating
             identity matmuls per 128-row chunk land TRANSPOSED fp32 dots
             in PSUM (fp16 PSUM transposes do NOT accumulate on TRN2; real
             fp32 matmuls do); ACT exp (scale folded).
  exback(g-1): two PE fp16 transposes + ACT copy bring exp'd dots back to
             token-major ex4.
  sm(g-2):   DVE softmax tail (den/recip/attn-normalize) + numerator
             multiply -> pn [t, h, c, g8].
  out(g-3):  the g-sum folds via 8 parity-accumulating fp32 identity
             matmuls per chunk (stride-8 stationary APs) while transposing
             pn to channel-major; ACT evacuates; PE out-projection
             (contraction 256, wox = Wo^T unexpanded); ACT bias; DMA store.
Engine balance (sim, hot): DVE ~377us, PE ~360us, ACT ~308us, DMA ~141us.
GpSimd is deliberately idle: measured Q7 launch overhead is ~2-5us per op
and heavy Pool traffic degrades concurrent DVE adds (SBUF contention).
DVE TensorTensor runs in 2x mode (fp16, packed innermost dims); all TT APs
must optimize to <= 3 free dims (broadcast (t,h)/(c,g) merges matter).
Custom DVE ops (e.g. reciprocal_approx_fast) crash this runtime
(NRT_EXEC_UNIT_UNRECOVERABLE) - stock ops only.
Measured: 446-447us HW exec (traced, core 0), rel err ~5.4e-4 vs fp32
reference; baseline at session start was 568-570us.
"""

import os
import sys

if "/opt/trn_rl_repo" not in sys.path:
    sys.path.insert(0, "/opt/trn_rl_repo")

from contextlib import ExitStack

import numpy as np

import concourse.bacc as bacc
import concourse.tile as tile
from concourse import mybir
from concourse.bass_utils import run_bass_kernel_spmd
from concourse.masks import make_identity

B, C, HH, WW = 8, 256, 128, 128
NH, HD = 8, 32
N = HH * WW  # tokens per sample
TT = 128  # tokens per sub-tile (partition dim)
G = 4  # sub-tiles per group
GT = G * TT
GR = 1  # residual g extent entering the out-projection (PE folds 8 -> GR)
KCH = (C * GR) // 128  # contraction chunks for the out-projection
SCALE = float(HD) ** -0.5

PRESUM = "pool16"  # x-sum: fp16 ACT casts + one cheap fp16 Pool add
GRAM_TILES = 4  # per-sub-tile gram ops (keeps every AP <= 3 free dims)

F32 = mybir.dt.float32
F16 = mybir.dt.float16
AX = mybir.AxisListType


def build_kernel(n_tiles=N // TT, has_qkv_bias=False, has_o_bias=False):
    assert n_tiles % G == 0
    n_groups = n_tiles // G
    nc = bacc.Bacc(trn_type="TRN2")

    xc = nc.declare_dram_parameter("xc", [C, N], F32, isOutput=False)
    xt = nc.declare_dram_parameter("xt", [C, N], F32, isOutput=False)
    wqkv = nc.declare_dram_parameter("wqkv", [C, 3 * C], F16, isOutput=False)
    wox = nc.declare_dram_parameter("wox", [C * GR, C], F16, isOutput=False)
    bqkv = nc.declare_dram_parameter("bqkv", [1, 3 * C], F16, isOutput=False)
    bo = nc.declare_dram_parameter("bo", [C, 1], F32, isOutput=False)
    out = nc.declare_dram_parameter("out", [C, N], F32, isOutput=True)

    with tile.TileContext(nc) as tc, ExitStack() as ctx:
        singles = ctx.enter_context(tc.tile_pool(name="singles", bufs=1))
        xs_pool = ctx.enter_context(tc.tile_pool(name="xs", bufs=2))
        x16_pool = ctx.enter_context(tc.tile_pool(name="x16", bufs=2))
        qkv_pool = ctx.enter_context(tc.tile_pool(name="qkv", bufs=5))
        gp_pool = ctx.enter_context(tc.tile_pool(name="gp", bufs=2))
        pn_pool = ctx.enter_context(tc.tile_pool(name="pn", bufs=3))
        tree_pool = ctx.enter_context(tc.tile_pool(name="tree", bufs=1))
        tree2_pool = ctx.enter_context(tc.tile_pool(name="tree2", bufs=2))
        sm_pool = ctx.enter_context(tc.tile_pool(name="sm", bufs=4))
        otr_pool = ctx.enter_context(tc.tile_pool(name="otr", bufs=2))
        out_pool = ctx.enter_context(tc.tile_pool(name="outp", bufs=2))
        ps_qkv = ctx.enter_context(tc.tile_pool(name="ps_qkv", bufs=2, space="PSUM"))
        ps_tr = ctx.enter_context(tc.tile_pool(name="ps_tr", bufs=2, space="PSUM"))
        ps_out = ctx.enter_context(tc.tile_pool(name="ps_out", bufs=1, space="PSUM"))
        ps_dots = ctx.enter_context(tc.tile_pool(name="ps_dots", bufs=1, space="PSUM"))
        ps_back = ctx.enter_context(tc.tile_pool(name="ps_back", bufs=1, space="PSUM"))

        wqkv_sb = singles.tile([128, 2, 3 * C], F16)
        wox_sb = singles.tile([128, KCH, C], F16)
        ident = singles.tile([128, 128], F16)
        if has_qkv_bias:
            bqkv_sb = singles.tile([1, 3 * C], F16)
            ones_tok = singles.tile([1, TT], F16)
        if has_o_bias:
            bo_sb = singles.tile([128, 2], F32)

        def emit_singles():
            nc.sync.dma_start(
                out=wqkv_sb, in_=wqkv.rearrange("(b p) m -> p b m", b=2)
            )
            nc.sync.dma_start(
                out=wox_sb, in_=wox.rearrange("(k p) m -> p k m", k=KCH)
            )
            make_identity(nc, ident)
            if has_qkv_bias:
                nc.sync.dma_start(out=bqkv_sb, in_=bqkv)
                nc.vector.memset(ones_tok, 1.0)
            if has_o_bias:
                nc.sync.dma_start(
                    out=bo_sb, in_=bo.rearrange("(b p) o -> p (b o)", b=2)
                )

        xc_r = xc.rearrange("(b p) n -> p b n", b=2)
        xt_r = xt.rearrange("(b p) n -> p b n", b=2)
        out_r = out.rearrange("(b p) n -> b p n", b=2)

        st = {}  # per-group state handed between phases

        def emit_load(g):
            gtok = slice(g * GT, (g + 1) * GT)
            xc_t = xs_pool.tile([128, 2, GT], F32, tag="xc")
            xt_t = xs_pool.tile([128, 2, GT], F32, tag="xt")
            xc16 = x16_pool.tile([128, 2, GT], F16, tag="xc16")
            xt16 = x16_pool.tile([128, 2, GT], F16, tag="xt16")
            x16 = x16_pool.tile([128, 2, GT], F16, tag="x16")
            if g == 0:
                for j in range(G):
                    jt = slice(j * TT, (j + 1) * TT)
                    gjt = slice(g * GT + j * TT, g * GT + (j + 1) * TT)
                    nc.sync.dma_start(out=xc_t[:, :, jt], in_=xc_r[:, :, gjt])
                    nc.sync.dma_start(out=xt_t[:, :, jt], in_=xt_r[:, :, gjt])
                    nc.scalar.copy(out=xc16[:, :, jt], in_=xc_t[:, :, jt])
                    nc.scalar.copy(out=xt16[:, :, jt], in_=xt_t[:, :, jt])
                    nc.gpsimd.tensor_add(
                        x16[:, :, jt], xc16[:, :, jt], xt16[:, :, jt]
                    )
            else:
                nc.sync.dma_start(out=xc_t, in_=xc_r[:, :, gtok])
                nc.sync.dma_start(out=xt_t, in_=xt_r[:, :, gtok])
                nc.scalar.copy(out=xc16, in_=xc_t)
                nc.scalar.copy(out=xt16, in_=xt_t)
                nc.gpsimd.tensor_add(x16, xc16, xt16)
            st[g] = {"x16": x16, "gtok": gtok}

        def emit_proj(g):
            x16 = st[g].pop("x16")
            x_srcs = (x16,)
            qkv = qkv_pool.tile([128, G, 3, C], F16, tag="qkv")
            gp = gp_pool.tile([128, G, NH, NH, HD], F16, tag="gp")

            gram_tiles = G if g == 0 else GRAM_TILES
            gstep = G // gram_tiles

            def emit_gram(t0, tn):
                q_b = (
                    qkv[:, t0:tn, 0]
                    .rearrange("p t (h c) -> p t h c", h=NH)
                    .unsqueeze(3)
                    .broadcast_to([TT, tn - t0, NH, NH, HD])
                )
                k_b = (
                    qkv[:, t0:tn, 1]
                    .rearrange("p t (g c) -> p t g c", g=NH)
                    .unsqueeze(2)
                    .broadcast_to([TT, tn - t0, NH, NH, HD])
                )
                nc.vector.tensor_mul(gp[:, t0:tn], q_b, k_b)

            # per sub-tile qkv projection via PSUM accumulation over
            # channel-half b; gram multiply emitted as soon as its input
            # sub-tiles land so the DVE never waits on the full group.
            n_acc = 2 * len(x_srcs) + (1 if has_qkv_bias else 0)
            qkv_flat = qkv.rearrange("p t r m -> p t (r m)")
            for j in range(G):
                for lo, hi in ((0, 384), (384, 768)):
                    qkv_ps = ps_qkv.tile([TT, 384], F32, tag="qkvps")
                    acc = 0
                    for xsrc in x_srcs:
                        for b in range(2):
                            nc.tensor.matmul(
                                qkv_ps,
                                lhsT=xsrc[:, b, j * TT : (j + 1) * TT],
                                rhs=wqkv_sb[:, b, lo:hi],
                                start=(acc == 0),
                                stop=(acc == n_acc - 1),
                            )
                            acc += 1
                    if has_qkv_bias:
                        nc.tensor.matmul(
                            qkv_ps,
                            lhsT=ones_tok,
                            rhs=bqkv_sb[:, lo:hi],
                            start=False,
                            stop=True,
                        )
                    nc.scalar.copy(out=qkv_flat[:, j, lo:hi], in_=qkv_ps)
                if (j + 1) % gstep == 0:
                    emit_gram(j + 1 - gstep, j + 1)

            # gram c-tree: one DVE level (c 32 -> 16); the rest of the
            # reduction rides PE (parity-accumulating matmuls)
            t16 = tree2_pool.tile([128, G * 64, 16], F16, tag="t16")
            tree_slices = (
                [slice(j * 64, (j + 1) * 64) for j in range(G)]
                if g == 0
                else [slice(0, G * 64)]
            )
            gpv = gp.rearrange("p t h g c -> p (t h g) c")  # [128, 256, 32]
            for sl in tree_slices:
                nc.vector.tensor_add(t16[:, sl], gpv[:, sl, 0:16], gpv[:, sl, 16:32])
            st[g]["qkv"] = qkv
            st[g]["t16"] = t16

        def emit_dots(g):
            # c-remnant 8 folded on PE into TRANSPOSED fp32 dots
            # [(rows=(tloc%2,h,g)), chunk=(tloc//2), t]; exp on ACT; two
            # fp16 transposes bring ex back to token-major for the softmax.
            t16 = st[g].pop("t16")
            dots_ps = ps_dots.tile([128, 2, TT], F32, tag="dots")
            for chunk in range(2):
                for par in range(16):
                    nc.tensor.matmul(
                        dots_ps[:, chunk],
                        lhsT=t16[:, chunk * 128 : (chunk + 1) * 128, par],
                        rhs=ident,
                        start=(par == 0),
                        stop=(par == 15),
                    )
            ex_t = sm_pool.tile([128, 2, TT], F16, tag="ex_t")
            nc.scalar.activation(
                out=ex_t.rearrange("p c t -> p (c t)"),
                in_=dots_ps.rearrange("p c t -> p (c t)"),
                func=mybir.ActivationFunctionType.Exp,
                scale=SCALE,
            )
            st[g]["ex_t"] = ex_t

        def emit_exback(g):
            ex_t = st[g].pop("ex_t")
            back_ps = ps_back.tile([128, 2, TT], F16, tag="back")
            for chunk in range(2):
                nc.tensor.transpose(back_ps[:, chunk], ex_t[:, chunk], ident)
            ex4 = sm_pool.tile([128, G, NH, NH], F16, tag="ex4")
            nc.scalar.copy(
                out=ex4.rearrange("p t h g -> p (t h g)"),
                in_=back_ps.rearrange("p c t -> p (c t)"),
            )
            st[g]["ex4"] = ex4

        def emit_sm(g):
            qkv = st[g].pop("qkv")
            ex4 = st[g].pop("ex4")
            # softmax tail (DVE, token-major)
            den4 = sm_pool.tile([128, G * NH], F32, tag="den4")
            nc.vector.reduce_sum(
                den4, ex4.rearrange("p t h g -> p (t h) g"), axis=AX.X
            )
            rec4 = sm_pool.tile([128, G * NH], F16, tag="rec4")
            with nc.allow_low_precision(reason="softmax weights tolerate fp16"):
                nc.vector.reciprocal(rec4, den4)
            attn4 = sm_pool.tile([128, G, NH, NH], F16, tag="attn4")
            nc.vector.tensor_mul(
                attn4.rearrange("p t h g -> p (t h) g"),
                ex4.rearrange("p t h g -> p (t h) g"),
                rec4.unsqueeze(2).broadcast_to([128, G * NH, NH]),
            )

            # numerator multiply: pn [t, h, c, g8] (contiguous layout --
            # all APs merge to <=3 free dims)
            pn = pn_pool.tile([128, G, NH, HD, NH], F16, tag="pn")
            vr = qkv[:, :, 2].rearrange("p t (c g) -> p t c g", g=NH)
            a_b = attn4.unsqueeze(3).broadcast_to([TT, G, NH, HD, NH])
            v_b = vr.unsqueeze(2).broadcast_to([TT, G, NH, HD, NH])
            nc.vector.tensor_mul(pn, a_b, v_b)
            st[g]["pn"] = pn

        def emit_out(g):
            pn = st[g].pop("pn")
            gtok = st[g].pop("gtok")
            del st[g]
            # per sub-tile: the g 8->4 fold rides fp32 PSUM accumulation on
            # PE via REAL matmuls against the identity (fp16 PSUM transposes
            # do not accumulate on TRN2, fp32 matmul PSUM does). Stride-2
            # stationary APs pick even/odd g parities; wox rows repeat WoT
            # per pair so any pairing is valid. Half-tile PSUM chunks keep
            # the PE->ACT evacuation pipelined within the 16KB PSUM budget.
            otr_sb = otr_pool.tile([128, KCH, GT], F16, tag="otrsb")
            for j in range(G):
                pnj = pn[:, j].rearrange("p h c g -> p (h c) g")
                tr_ps = ps_tr.tile([128, KCH, TT], F32, tag="trps")
                for ci in range(KCH):
                    for par in range(NH):
                        nc.tensor.matmul(
                            tr_ps[:, ci],
                            lhsT=pnj[:, ci * 128 : (ci + 1) * 128, par],
                            rhs=ident,
                            start=(par == 0),
                            stop=(par == NH - 1),
                        )
                nc.scalar.copy(
                    out=otr_sb[:, :, j * TT : (j + 1) * TT], in_=tr_ps
                )

            # out-projection with g-expanded Wo': contracts (h,c,g4)=1024
            # in KCH accumulating chunk-matmuls per 128-channel output half.
            out_ps = ps_out.tile([128, 2, GT], F32, tag="outps")
            for m in range(2):
                for ci in range(KCH):
                    nc.tensor.matmul(
                        out_ps[:, m],
                        lhsT=wox_sb[:, ci, m * 128 : (m + 1) * 128],
                        rhs=otr_sb[:, ci],
                        start=(ci == 0),
                        stop=(ci == KCH - 1),
                    )
            out_sb = out_pool.tile([128, 2, GT], F32, tag="outsb")
            if has_o_bias:
                for m in range(2):
                    nc.scalar.activation(
                        out=out_sb[:, m],
                        in_=out_ps[:, m],
                        func=mybir.ActivationFunctionType.Identity,
                        bias=bo_sb[:, m : m + 1],
                    )
            else:
                nc.scalar.copy(out=out_sb, in_=out_ps)
            for b in range(2):
                nc.sync.dma_start(out=out_r[b, :, gtok], in_=out_sb[:, b])

        emitters = {
            "load": emit_load,
            "proj": emit_proj,
            "dots": emit_dots,
            "exback": emit_exback,
            "sm": emit_sm,
            "out": emit_out,
        }
        # emission order + lags tuned so every engine's in-order queue sees
        # ops in dependency-ready order
        schedule = (
            ("load", 0), ("sm", 2), ("proj", 0), ("dots", 1),
            ("exback", 1), ("out", 3),
        )
        emit_singles()
        max_lag = max(lag for _, lag in schedule)
        for i in range(n_groups + max_lag):
            for phase, lag in schedule:
                gi = i - lag
                if 0 <= gi < n_groups:
                    emitters[phase](gi)

    nc.compile()
    return nc


# split_heads uses channel c*nh+h for (head h, dim c); permute projection rows
# so q,k come out head-major (h*32+c). v stays in natural order: its (c,g)
# interleave is exactly what the numerator multiply wants. The out-proj weight
# is expanded over g: pn layout is [h, c, g] so row (h*32+c)*8+g of Wo' equals
# WoT row h*32+c (Wo's input channels already match head-major merge order).
_PERM = np.array([c * NH + h for h in range(NH) for c in range(HD)])

# delta matrices for the PE-side softmax: rows r = (tloc%2)*64 + h*8 + g map
# to den/rec partitions po = (tloc%2)*8 + h
_R = np.arange(128)
_PO = (_R // 64) * 8 + (_R // 8) % 8
DG = np.zeros((128, 16), np.float16)
DG[_R, _PO] = 1.0
DR = np.ascontiguousarray(DG.T)


def _prep_weights(Wq, bq, Wk, bk, Wv, bv, Wo, bo):
    wqkv = np.concatenate([Wq[_PERM].T, Wk[_PERM].T, Wv.T], axis=1)
    wqkv = np.ascontiguousarray(wqkv).astype(np.float16)
    wot = np.ascontiguousarray(Wo.T)  # [256 (h,c), 256 m]
    wox = np.repeat(wot, GR, axis=0).astype(np.float16)  # [(h,c,gGR), 256]
    bqkv = np.concatenate([bq[_PERM], bk[_PERM], bv]).astype(np.float16)[None]
    bo_a = bo.astype(np.float32)[:, None]
    return wqkv, wox, bqkv, bo_a


def _in_maps(x_c, x_t, wqkv, wox, bqkv, bo_a):
    maps = []
    for b in range(B):
        maps.append(
            {
                "xc": np.ascontiguousarray(x_c[b].reshape(C, N)),
                "xt": np.ascontiguousarray(x_t[b].reshape(C, N)),
                "wqkv": wqkv,
                "wox": wox,
                "bqkv": bqkv,
                "bo": bo_a,
            }
        )
    return maps


def kernel(x_c, x_t, Wq, bq, Wk, bk, Wv, bv, Wo, bo):
    x_c = np.asarray(x_c, dtype=np.float32)
    x_t = np.asarray(x_t, dtype=np.float32)
    wqkv, wox, bqkv, bo_a = _prep_weights(
        np.asarray(Wq, np.float32),
        np.asarray(bq, np.float32),
        np.asarray(Wk, np.float32),
        np.asarray(bk, np.float32),
        np.asarray(Wv, np.float32),
        np.asarray(bv, np.float32),
        np.asarray(Wo, np.float32),
        np.asarray(bo, np.float32),
    )
    nc = build_kernel(
        has_qkv_bias=bool(np.any(bqkv)), has_o_bias=bool(np.any(bo_a))
    )
    res = run_bass_kernel_spmd(
        nc, _in_maps(x_c, x_t, wqkv, wox, bqkv, bo_a), list(range(B))
    )
    outs = [np.asarray(res.results[b]["out"]).reshape(C, HH, WW) for b in range(B)]
    return np.stack(outs).astype(np.float32)


def _install_ntff_shim():
    """Recreate the missing antenv.axon_hooks module + ctypes NTFF hook
    (mirrors trn_agent_boot.trn_boot's degraded-silently path). Test-only."""
    import contextlib
    import ctypes
    import types

    try:
        from antenv.axon_hooks import get_axon_ntff_profile_hook  # noqa: F401

        return True
    except ImportError:
        pass
    import antenv

    mod = types.ModuleType("antenv.axon_hooks")
    mod._hook = None

    def set_axon_ntff_profile_hook(h):
        mod._hook = h

    def get_axon_ntff_profile_hook():
        return mod._hook

    mod.set_axon_ntff_profile_hook = set_axon_ntff_profile_hook
    mod.get_axon_ntff_profile_hook = get_axon_ntff_profile_hook
    sys.modules["antenv.axon_hooks"] = mod
    antenv.axon_hooks = mod

    so_path = "/opt/axon/libaxon_pjrt.so"
    if not os.path.exists(so_path):
        return False
    lib = ctypes.CDLL(so_path)
    if not hasattr(lib, "axon_start_nrt_profile"):
        return False
    lib.axon_start_nrt_profile.argtypes = [
        ctypes.POINTER(ctypes.c_int64),
        ctypes.c_size_t,
    ]
    lib.axon_start_nrt_profile.restype = ctypes.c_int64
    lib.axon_stop_nrt_profile.argtypes = [ctypes.c_char_p]
    lib.axon_stop_nrt_profile.restype = ctypes.c_int64

    @contextlib.contextmanager
    def _hook(output_dir, device_ids):
        import jax

        jax.devices()
        if device_ids:
            ids = (ctypes.c_int64 * len(device_ids))(*device_ids)
            rc = lib.axon_start_nrt_profile(ids, len(device_ids))
        else:
            rc = lib.axon_start_nrt_profile(None, 0)
        if rc != 0:
            raise RuntimeError(f"axon_start_nrt_profile rc={rc}")
        try:
            yield
        finally:
            n = lib.axon_stop_nrt_profile(str(output_dir).encode())
            print(f"profile: {n} file(s) written to {output_dir}")

    set_axon_ntff_profile_hook(_hook)
    return True


def profile_run(inputs_np):
    """Run once more with NTFF tracing on core 0; return exec_time_ns."""
    import concourse.bass_utils as bu

    _install_ntff_shim()
    bu.upload_artifacts = lambda d: "local://" + d  # no S3 in this container
    x_c = np.asarray(inputs_np["x_c"], np.float32)
    x_t = np.asarray(inputs_np["x_t"], np.float32)
    wqkv, wox, bqkv, bo_a = _prep_weights(
        *[
            np.asarray(inputs_np[k], np.float32)
            for k in ("Wq", "bq", "Wk", "bk", "Wv", "bv", "Wo", "bo")
        ]
    )
    nc = build_kernel(
        has_qkv_bias=bool(np.any(bqkv)), has_o_bias=bool(np.any(bo_a))
    )
    res = run_bass_kernel_spmd(
        nc,
        _in_maps(x_c, x_t, wqkv, wox, bqkv, bo_a),
        list(range(B)),
        trace=True,
    )
    return res.exec_time_ns


if __name__ == "__main__":
    rng = np.random.default_rng(0)
    ins = {
        "x_c": rng.standard_normal((B, C, HH, WW), dtype=np.float32),
        "x_t": rng.standard_normal((B, C, HH, WW), dtype=np.float32),
        "Wq": (rng.standard_normal((C, C)) * 0.02).astype(np.float32),
        "bq": np.zeros(C, np.float32),
        "Wk": (rng.standard_normal((C, C)) * 0.02).astype(np.float32),
        "bk": np.zeros(C, np.float32),
        "Wv": (rng.standard_normal((C, C)) * 0.02).astype(np.float32),
        "bv": np.zeros(C, np.float32),
        "Wo": (rng.standard_normal((C, C)) * 0.02).astype(np.float32),
        "bo": np.zeros(C, np.float32),
    }
    out = kernel(**ins)
    print(out.shape, out.dtype)


# revision 66
# speedup vs baseline: 1.0772x; 1.0772x over previous
"""Trainium2 Bass kernel: fused cross-head attention block (data parallel, 8 cores).

Problem (hardcoded shapes): x_c, x_t [8, 256, 128, 128] f32; Wq/Wk/Wv/Wo
[256, 256]; biases [256]; out [8, 256, 128, 128] f32.

Math per sample (C=256, nh=8, hd=32, N=H*W=16384 tokens):
  x = x_c + x_t;  q/k/v = per-token linear projections of x
  per token: dot[h,g] = q_h . k_g * hd^-0.5   (8x8 gram ACROSS heads)
             attn = softmax_g(dot);  o_h = sum_g attn[h,g] v_g
  out = Wo @ o (+ bo)

Sharding: pure data parallel - one sample per NeuronCore, weights replicated.

Per core: 32 groups of 4x128-token tiles, software-pipelined in 6 phases
(lags in groups) so every in-order engine queue sees work dependency-ready:
  load(g):   DMA xc/xt; fp16 casts on ACT (the x-sum rides PE's PSUM
             accumulation over both sources in the qkv matmuls).
  proj(g):   PE qkv matmuls (384-wide PSUM half-tiles, dual-source + c-half
             accumulated); ACT evacuation; DVE per-tile gram multiply
             (broadcast APs) + ONE c-tree level (32 -> 16).
  dots(g-1): the remaining c-reduction rides PE: 16 parity-accumulating
             identity matmuls per 128-row chunk land TRANSPOSED fp32 dots
             in PSUM (fp16 PSUM transposes do NOT accumulate on TRN2; real
             fp32 matmuls do); ACT exp (scale folded).
  exback(g-1): two PE fp16 transposes + ACT copy bring exp'd dots back to
             token-major ex4.
  sm(g-2):   DVE softmax tail (den/recip/attn-normalize) + numerator
             multiply -> pn [t, h, c, g8].
  out(g-3):  the g-sum folds via 8 parity-accumulating fp32 identity
             matmuls per chunk (stride-8 stationary APs) while transposing
             pn to channel-major; ACT evacuates; PE out-projection
             (contraction 256, wox = Wo^T unexpanded); ACT bias; DMA store.
Engine balance (sim, hot): DVE ~377us, PE ~360us, ACT ~308us, DMA ~141us.
GpSimd is deliberately idle: measured Q7 launch overhead is ~2-5us per op
and heavy Pool traffic degrades concurrent DVE adds (SBUF contention).
DVE TensorTensor runs in 2x mode (fp16, packed innermost dims); all TT APs
must optimize to <= 3 free dims (broadcast (t,h)/(c,g) merges matter).
Custom DVE ops (e.g. reciprocal_approx_fast) crash this runtime
(NRT_EXEC_UNIT_UNRECOVERABLE) - stock ops only.
Measured: 446-447us HW exec (traced, core 0), rel err ~5.4e-4 vs fp32
reference; baseline at session start was 568-570us.
"""

import os
import sys

if "/opt/trn_rl_repo" not in sys.path:
    sys.path.insert(0, "/opt/trn_rl_repo")

from contextlib import ExitStack

import numpy as np

import concourse.bacc as bacc
import concourse.tile as tile
from concourse import mybir
from concourse.bass_utils import run_bass_kernel_spmd
from concourse.masks import make_identity

B, C, HH, WW = 8, 256, 128, 128
NH, HD = 8, 32
N = HH * WW  # tokens per sample
TT = 128  # tokens per sub-tile (partition dim)
G = 4  # sub-tiles per group
GT = G * TT
GR = 1  # residual g extent entering the out-projection (PE folds 8 -> GR)
KCH = (C * GR) // 128  # contraction chunks for the out-projection
SCALE = float(HD) ** -0.5

PRESUM = "pe"  # x-sum rides dual-source PSUM accumulation on PE
GRAM_TILES = 4  # per-sub-tile gram ops (keeps every AP <= 3 free dims)

F32 = mybir.dt.float32
F16 = mybir.dt.float16
AX = mybir.AxisListType


def build_kernel(n_tiles=N // TT, has_qkv_bias=False, has_o_bias=False):
    assert n_tiles % G == 0
    n_groups = n_tiles // G
    nc = bacc.Bacc(trn_type="TRN2")

    xc = nc.declare_dram_parameter("xc", [C, N], F32, isOutput=False)
    xt = nc.declare_dram_parameter("xt", [C, N], F32, isOutput=False)
    wqkv = nc.declare_dram_parameter("wqkv", [C, 3 * C], F16, isOutput=False)
    wox = nc.declare_dram_parameter("wox", [C * GR, C], F16, isOutput=False)
    bqkv = nc.declare_dram_parameter("bqkv", [1, 3 * C], F16, isOutput=False)
    bo = nc.declare_dram_parameter("bo", [C, 1], F32, isOutput=False)
    out = nc.declare_dram_parameter("out", [C, N], F32, isOutput=True)

    with tile.TileContext(nc) as tc, ExitStack() as ctx:
        singles = ctx.enter_context(tc.tile_pool(name="singles", bufs=1))
        xs_pool = ctx.enter_context(tc.tile_pool(name="xs", bufs=2))
        x16_pool = ctx.enter_context(tc.tile_pool(name="x16", bufs=2))
        qkv_pool = ctx.enter_context(tc.tile_pool(name="qkv", bufs=5))
        gp_pool = ctx.enter_context(tc.tile_pool(name="gp", bufs=2))
        pn_pool = ctx.enter_context(tc.tile_pool(name="pn", bufs=3))
        tree_pool = ctx.enter_context(tc.tile_pool(name="tree", bufs=1))
        tree2_pool = ctx.enter_context(tc.tile_pool(name="tree2", bufs=2))
        sm_pool = ctx.enter_context(tc.tile_pool(name="sm", bufs=4))
        otr_pool = ctx.enter_context(tc.tile_pool(name="otr", bufs=2))
        out_pool = ctx.enter_context(tc.tile_pool(name="outp", bufs=2))
        ps_qkv = ctx.enter_context(tc.tile_pool(name="ps_qkv", bufs=2, space="PSUM"))
        ps_tr = ctx.enter_context(tc.tile_pool(name="ps_tr", bufs=2, space="PSUM"))
        ps_out = ctx.enter_context(tc.tile_pool(name="ps_out", bufs=1, space="PSUM"))
        ps_dots = ctx.enter_context(tc.tile_pool(name="ps_dots", bufs=1, space="PSUM"))
        ps_back = ctx.enter_context(tc.tile_pool(name="ps_back", bufs=1, space="PSUM"))

        wqkv_sb = singles.tile([128, 2, 3 * C], F16)
        wox_sb = singles.tile([128, KCH, C], F16)
        ident = singles.tile([128, 128], F16)
        if has_qkv_bias:
            bqkv_sb = singles.tile([1, 3 * C], F16)
            ones_tok = singles.tile([1, TT], F16)
        if has_o_bias:
            bo_sb = singles.tile([128, 2], F32)

        def emit_singles():
            nc.sync.dma_start(
                out=wqkv_sb, in_=wqkv.rearrange("(b p) m -> p b m", b=2)
            )
            nc.sync.dma_start(
                out=wox_sb, in_=wox.rearrange("(k p) m -> p k m", k=KCH)
            )
            make_identity(nc, ident)
            if has_qkv_bias:
                nc.sync.dma_start(out=bqkv_sb, in_=bqkv)
                nc.vector.memset(ones_tok, 1.0)
            if has_o_bias:
                nc.sync.dma_start(
                    out=bo_sb, in_=bo.rearrange("(b p) o -> p (b o)", b=2)
                )

        xc_r = xc.rearrange("(b p) n -> p b n", b=2)
        xt_r = xt.rearrange("(b p) n -> p b n", b=2)
        out_r = out.rearrange("(b p) n -> b p n", b=2)

        st = {}  # per-group state handed between phases

        def emit_load(g):
            gtok = slice(g * GT, (g + 1) * GT)
            xc_t = xs_pool.tile([128, 2, GT], F32, tag="xc")
            xt_t = xs_pool.tile([128, 2, GT], F32, tag="xt")
            xc16 = x16_pool.tile([128, 2, GT], F16, tag="xc16")
            xt16 = x16_pool.tile([128, 2, GT], F16, tag="xt16")
            if g == 0:
                for j in range(G):
                    jt = slice(j * TT, (j + 1) * TT)
                    gjt = slice(g * GT + j * TT, g * GT + (j + 1) * TT)
                    nc.sync.dma_start(out=xc_t[:, :, jt], in_=xc_r[:, :, gjt])
                    nc.sync.dma_start(out=xt_t[:, :, jt], in_=xt_r[:, :, gjt])
                    nc.scalar.copy(out=xc16[:, :, jt], in_=xc_t[:, :, jt])
                    nc.scalar.copy(out=xt16[:, :, jt], in_=xt_t[:, :, jt])
            else:
                nc.sync.dma_start(out=xc_t, in_=xc_r[:, :, gtok])
                nc.sync.dma_start(out=xt_t, in_=xt_r[:, :, gtok])
                nc.scalar.copy(out=xc16, in_=xc_t)
                nc.scalar.copy(out=xt16, in_=xt_t)
            st[g] = {"x16": (xc16, xt16), "gtok": gtok}

        def emit_proj(g):
            x_srcs = st[g].pop("x16")
            qkv = qkv_pool.tile([128, G, 3, C], F16, tag="qkv")
            gp = gp_pool.tile([128, G, NH, NH, HD], F16, tag="gp")

            gram_tiles = G if g == 0 else GRAM_TILES
            gstep = G // gram_tiles

            def emit_gram(t0, tn):
                q_b = (
                    qkv[:, t0:tn, 0]
                    .rearrange("p t (h c) -> p t h c", h=NH)
                    .unsqueeze(3)
                    .broadcast_to([TT, tn - t0, NH, NH, HD])
                )
                k_b = (
                    qkv[:, t0:tn, 1]
                    .rearrange("p t (g c) -> p t g c", g=NH)
                    .unsqueeze(2)
                    .broadcast_to([TT, tn - t0, NH, NH, HD])
                )
                nc.vector.tensor_mul(gp[:, t0:tn], q_b, k_b)

            # per sub-tile qkv projection via PSUM accumulation over
            # channel-half b; gram multiply emitted as soon as its input
            # sub-tiles land so the DVE never waits on the full group.
            n_acc = 2 * len(x_srcs) + (1 if has_qkv_bias else 0)
            qkv_flat = qkv.rearrange("p t r m -> p t (r m)")
            for j in range(G):
                for lo, hi in ((0, 384), (384, 768)):
                    qkv_ps = ps_qkv.tile([TT, 384], F32, tag="qkvps")
                    acc = 0
                    for xsrc in x_srcs:
                        for b in range(2):
                            nc.tensor.matmul(
                                qkv_ps,
                                lhsT=xsrc[:, b, j * TT : (j + 1) * TT],
                                rhs=wqkv_sb[:, b, lo:hi],
                                start=(acc == 0),
                                stop=(acc == n_acc - 1),
                            )
                            acc += 1
                    if has_qkv_bias:
                        nc.tensor.matmul(
                            qkv_ps,
                            lhsT=ones_tok,
                            rhs=bqkv_sb[:, lo:hi],
                            start=False,
                            stop=True,
                        )
                    nc.scalar.copy(out=qkv_flat[:, j, lo:hi], in_=qkv_ps)
                if (j + 1) % gstep == 0:
                    emit_gram(j + 1 - gstep, j + 1)

            # gram c-tree: one DVE level (c 32 -> 16); the rest of the
            # reduction rides PE (parity-accumulating matmuls)
            t16 = tree2_pool.tile([128, G * 64, 16], F16, tag="t16")
            tree_slices = (
                [slice(j * 64, (j + 1) * 64) for j in range(G)]
                if g == 0
                else [slice(0, G * 64)]
            )
            gpv = gp.rearrange("p t h g c -> p (t h g) c")  # [128, 256, 32]
            for sl in tree_slices:
                nc.vector.tensor_add(t16[:, sl], gpv[:, sl, 0:16], gpv[:, sl, 16:32])
            st[g]["qkv"] = qkv
            st[g]["t16"] = t16

        def emit_dots(g):
            # c-remnant 8 folded on PE into TRANSPOSED fp32 dots
            # [(rows=(tloc%2,h,g)), chunk=(tloc//2), t]; exp on ACT; two
            # fp16 transposes bring ex back to token-major for the softmax.
            t16 = st[g].pop("t16")
            dots_ps = ps_dots.tile([128, 2, TT], F32, tag="dots")
            for chunk in range(2):
                for par in range(16):
                    nc.tensor.matmul(
                        dots_ps[:, chunk],
                        lhsT=t16[:, chunk * 128 : (chunk + 1) * 128, par],
                        rhs=ident,
                        start=(par == 0),
                        stop=(par == 15),
                    )
            ex_t = sm_pool.tile([128, 2, TT], F16, tag="ex_t")
            nc.scalar.activation(
                out=ex_t.rearrange("p c t -> p (c t)"),
                in_=dots_ps.rearrange("p c t -> p (c t)"),
                func=mybir.ActivationFunctionType.Exp,
                scale=SCALE,
            )
            st[g]["ex_t"] = ex_t

        def emit_exback(g):
            ex_t = st[g].pop("ex_t")
            back_ps = ps_back.tile([128, 2, TT], F16, tag="back")
            for chunk in range(2):
                nc.tensor.transpose(back_ps[:, chunk], ex_t[:, chunk], ident)
            ex4 = sm_pool.tile([128, G, NH, NH], F16, tag="ex4")
            nc.scalar.copy(
                out=ex4.rearrange("p t h g -> p (t h g)"),
                in_=back_ps.rearrange("p c t -> p (c t)"),
            )
            st[g]["ex4"] = ex4

        def emit_sm(g):
            qkv = st[g].pop("qkv")
            ex4 = st[g].pop("ex4")
            # softmax tail (DVE, token-major)
            den4 = sm_pool.tile([128, G * NH], F32, tag="den4")
            nc.vector.reduce_sum(
                den4, ex4.rearrange("p t h g -> p (t h) g"), axis=AX.X
            )
            rec4 = sm_pool.tile([128, G * NH], F16, tag="rec4")
            with nc.allow_low_precision(reason="softmax weights tolerate fp16"):
                nc.vector.reciprocal(rec4, den4)
            attn4 = sm_pool.tile([128, G, NH, NH], F16, tag="attn4")
            nc.vector.tensor_mul(
                attn4.rearrange("p t h g -> p (t h) g"),
                ex4.rearrange("p t h g -> p (t h) g"),
                rec4.unsqueeze(2).broadcast_to([128, G * NH, NH]),
            )

            # numerator multiply: pn [t, h, c, g8] (contiguous layout --
            # all APs merge to <=3 free dims)
            pn = pn_pool.tile([128, G, NH, HD, NH], F16, tag="pn")
            vr = qkv[:, :, 2].rearrange("p t (c g) -> p t c g", g=NH)
            a_b = attn4.unsqueeze(3).broadcast_to([TT, G, NH, HD, NH])
            v_b = vr.unsqueeze(2).broadcast_to([TT, G, NH, HD, NH])
            nc.vector.tensor_mul(pn, a_b, v_b)
            st[g]["pn"] = pn

        def emit_out(g):
            pn = st[g].pop("pn")
            gtok = st[g].pop("gtok")
            del st[g]
            # per sub-tile: the g 8->4 fold rides fp32 PSUM accumulation on
            # PE via REAL matmuls against the identity (fp16 PSUM transposes
            # do not accumulate on TRN2, fp32 matmul PSUM does). Stride-2
            # stationary APs pick even/odd g parities; wox rows repeat WoT
            # per pair so any pairing is valid. Half-tile PSUM chunks keep
            # the PE->ACT evacuation pipelined within the 16KB PSUM budget.
            otr_sb = otr_pool.tile([128, KCH, GT], F16, tag="otrsb")
            for j in range(G):
                pnj = pn[:, j].rearrange("p h c g -> p (h c) g")
                tr_ps = ps_tr.tile([128, KCH, TT], F32, tag="trps")
                for ci in range(KCH):
                    for par in range(NH):
                        nc.tensor.matmul(
                            tr_ps[:, ci],
                            lhsT=pnj[:, ci * 128 : (ci + 1) * 128, par],
                            rhs=ident,
                            start=(par == 0),
                            stop=(par == NH - 1),
                        )
                nc.scalar.copy(
                    out=otr_sb[:, :, j * TT : (j + 1) * TT], in_=tr_ps
                )

            # out-projection with g-expanded Wo': contracts (h,c,g4)=1024
            # in KCH accumulating chunk-matmuls per 128-channel output half.
            out_ps = ps_out.tile([128, 2, GT], F32, tag="outps")
            for m in range(2):
                for ci in range(KCH):
                    nc.tensor.matmul(
                        out_ps[:, m],
                        lhsT=wox_sb[:, ci, m * 128 : (m + 1) * 128],
                        rhs=otr_sb[:, ci],
                        start=(ci == 0),
                        stop=(ci == KCH - 1),
                    )
            out_sb = out_pool.tile([128, 2, GT], F32, tag="outsb")
            if has_o_bias:
                for m in range(2):
                    nc.scalar.activation(
                        out=out_sb[:, m],
                        in_=out_ps[:, m],
                        func=mybir.ActivationFunctionType.Identity,
                        bias=bo_sb[:, m : m + 1],
                    )
            else:
                nc.scalar.copy(out=out_sb, in_=out_ps)
            for b in range(2):
                nc.sync.dma_start(out=out_r[b, :, gtok], in_=out_sb[:, b])

        emitters = {
            "load": emit_load,
            "proj": emit_proj,
            "dots": emit_dots,
            "exback": emit_exback,
            "sm": emit_sm,
            "out": emit_out,
        }
        # emission order + lags tuned so every engine's in-order queue sees
        # ops in dependency-ready order
        schedule = (
            ("load", 0), ("sm", 2), ("proj", 0), ("dots", 1),
            ("exback", 1), ("out", 3),
        )
        emit_singles()
        max_lag = max(lag for _, lag in schedule)
        for i in range(n_groups + max_lag):
            for phase, lag in schedule:
                gi = i - lag
                if 0 <= gi < n_groups:
                    emitters[phase](gi)

    nc.compile()
    return nc


# split_heads uses channel c*nh+h for (head h, dim c); permute projection rows
# so q,k come out head-major (h*32+c). v stays in natural order: its (c,g)
# interleave is exactly what the numerator multiply wants. The out-proj weight
# is expanded over g: pn layout is [h, c, g] so row (h*32+c)*8+g of Wo' equals
# WoT row h*32+c (Wo's input channels already match head-major merge order).
_PERM = np.array([c * NH + h for h in range(NH) for c in range(HD)])

# delta matrices for the PE-side softmax: rows r = (tloc%2)*64 + h*8 + g map
# to den/rec partitions po = (tloc%2)*8 + h
_R = np.arange(128)
_PO = (_R // 64) * 8 + (_R // 8) % 8
DG = np.zeros((128, 16), np.float16)
DG[_R, _PO] = 1.0
DR = np.ascontiguousarray(DG.T)


def _prep_weights(Wq, bq, Wk, bk, Wv, bv, Wo, bo):
    wqkv = np.concatenate([Wq[_PERM].T, Wk[_PERM].T, Wv.T], axis=1)
    wqkv = np.ascontiguousarray(wqkv).astype(np.float16)
    wot = np.ascontiguousarray(Wo.T)  # [256 (h,c), 256 m]
    wox = np.repeat(wot, GR, axis=0).astype(np.float16)  # [(h,c,gGR), 256]
    bqkv = np.concatenate([bq[_PERM], bk[_PERM], bv]).astype(np.float16)[None]
    bo_a = bo.astype(np.float32)[:, None]
    return wqkv, wox, bqkv, bo_a


def _in_maps(x_c, x_t, wqkv, wox, bqkv, bo_a):
    maps = []
    for b in range(B):
        maps.append(
            {
                "xc": np.ascontiguousarray(x_c[b].reshape(C, N)),
                "xt": np.ascontiguousarray(x_t[b].reshape(C, N)),
                "wqkv": wqkv,
                "wox": wox,
                "bqkv": bqkv,
                "bo": bo_a,
            }
        )
    return maps


def kernel(x_c, x_t, Wq, bq, Wk, bk, Wv, bv, Wo, bo):
    x_c = np.asarray(x_c, dtype=np.float32)
    x_t = np.asarray(x_t, dtype=np.float32)
    wqkv, wox, bqkv, bo_a = _prep_weights(
        np.asarray(Wq, np.float32),
        np.asarray(bq, np.float32),
        np.asarray(Wk, np.float32),
        np.asarray(bk, np.float32),
        np.asarray(Wv, np.float32),
        np.asarray(bv, np.float32),
        np.asarray(Wo, np.float32),
        np.asarray(bo, np.float32),
    )
    nc = build_kernel(
        has_qkv_bias=bool(np.any(bqkv)), has_o_bias=bool(np.any(bo_a))
    )
    res = run_bass_kernel_spmd(
        nc, _in_maps(x_c, x_t, wqkv, wox, bqkv, bo_a), list(range(B))
    )
    outs = [np.asarray(res.results[b]["out"]).reshape(C, HH, WW) for b in range(B)]
    return np.stack(outs).astype(np.float32)


def _install_ntff_shim():
    """Recreate the missing antenv.axon_hooks module + ctypes NTFF hook
    (mirrors trn_agent_boot.trn_boot's degraded-silently path). Test-only."""
    import contextlib
    import ctypes
    import types

    try:
        from antenv.axon_hooks import get_axon_ntff_profile_hook  # noqa: F401

        return True
    except ImportError:
        pass
    import antenv

    mod = types.ModuleType("antenv.axon_hooks")
    mod._hook = None

    def set_axon_ntff_profile_hook(h):
        mod._hook = h

    def get_axon_ntff_profile_hook():
        return mod._hook

    mod.set_axon_ntff_profile_hook = set_axon_ntff_profile_hook
    mod.get_axon_ntff_profile_hook = get_axon_ntff_profile_hook
    sys.modules["antenv.axon_hooks"] = mod
    antenv.axon_hooks = mod

    so_path = "/opt/axon/libaxon_pjrt.so"
    if not os.path.exists(so_path):
        return False
    lib = ctypes.CDLL(so_path)
    if not hasattr(lib, "axon_start_nrt_profile"):
        return False
    lib.axon_start_nrt_profile.argtypes = [
        ctypes.POINTER(ctypes.c_int64),
        ctypes.c_size_t,
    ]
    lib.axon_start_nrt_profile.restype = ctypes.c_int64
    lib.axon_stop_nrt_profile.argtypes = [ctypes.c_char_p]
    lib.axon_stop_nrt_profile.restype = ctypes.c_int64

    @contextlib.contextmanager
    def _hook(output_dir, device_ids):
        import jax

        jax.devices()
        if device_ids:
            ids = (ctypes.c_int64 * len(device_ids))(*device_ids)
            rc = lib.axon_start_nrt_profile(ids, len(device_ids))
        else:
            rc = lib.axon_start_nrt_profile(None, 0)
        if rc != 0:
            raise RuntimeError(f"axon_start_nrt_profile rc={rc}")
        try:
            yield
        finally:
            n = lib.axon_stop_nrt_profile(str(output_dir).encode())
            print(f"profile: {n} file(s) written to {output_dir}")

    set_axon_ntff_profile_hook(_hook)
    return True


def profile_run(inputs_np):
    """Run once more with NTFF tracing on core 0; return exec_time_ns."""
    import concourse.bass_utils as bu

    _install_ntff_shim()
    bu.upload_artifacts = lambda d: "local://" + d  # no S3 in this container
    x_c = np.asarray(inputs_np["x_c"], np.float32)
    x_t = np.asarray(inputs_np["x_t"], np.float32)
    wqkv, wox, bqkv, bo_a = _prep_weights(
        *[
            np.asarray(inputs_np[k], np.float32)
            for k in ("Wq", "bq", "Wk", "bk", "Wv", "bv", "Wo", "bo")
        ]
    )
    nc = build_kernel(
        has_qkv_bias=bool(np.any(bqkv)), has_o_bias=bool(np.any(bo_a))
    )
    res = run_bass_kernel_spmd(
        nc,
        _in_maps(x_c, x_t, wqkv, wox, bqkv, bo_a),
        list(range(B)),
        trace=True,
    )
    return res.exec_time_ns


if __name__ == "__main__":
    rng = np.random.default_rng(0)
    ins = {
        "x_c": rng.standard_normal((B, C, HH, WW), dtype=np.float32),
        "x_t": rng.standard_normal((B, C, HH, WW), dtype=np.float32),
        "Wq": (rng.standard_normal((C, C)) * 0.02).astype(np.float32),
        "bq": np.zeros(C, np.float32),
        "Wk": (rng.standard_normal((C, C)) * 0.02).astype(np.float32),
        "bk": np.zeros(C, np.float32),
        "Wv": (rng.standard_normal((C, C)) * 0.02).astype(np.float32),
        "bv": np.zeros(C, np.float32),
        "Wo": (rng.standard_normal((C, C)) * 0.02).astype(np.float32),
        "bo": np.zeros(C, np.float32),
    }
    out = kernel(**ins)
    print(out.shape, out.dtype)


# revision 69
# speedup vs baseline: 1.1283x; 1.0475x over previous
"""Trainium2 Bass kernel: fused cross-head attention block (data parallel, 8 cores).

Problem (hardcoded shapes): x_c, x_t [8, 256, 128, 128] f32; Wq/Wk/Wv/Wo
[256, 256]; biases [256]; out [8, 256, 128, 128] f32.

Math per sample (C=256, nh=8, hd=32, N=H*W=16384 tokens):
  x = x_c + x_t;  q/k/v = per-token linear projections of x
  per token: dot[h,g] = q_h . k_g * hd^-0.5   (8x8 gram ACROSS heads)
             attn = softmax_g(dot);  o_h = sum_g attn[h,g] v_g
  out = Wo @ o (+ bo)

Sharding: pure data parallel - one sample per NeuronCore, weights replicated.

Per core: 32 groups of 4x128-token tiles, software-pipelined in 6 phases
(lags in groups) so every in-order engine queue sees work dependency-ready:
  load(g):   DMA xc/xt; fp16 casts on ACT (the x-sum rides PE's PSUM
             accumulation over both sources in the qkv matmuls).
  proj(g):   PE qkv matmuls (384-wide PSUM half-tiles, dual-source + c-half
             accumulated); ACT evacuation; DVE per-tile gram multiply
             (broadcast APs) + ONE c-tree level (32 -> 16).
  dots(g-1): the remaining c-reduction rides PE: 16 parity-accumulating
             identity matmuls per 128-row chunk land TRANSPOSED fp32 dots
             in PSUM (fp16 PSUM transposes do NOT accumulate on TRN2; real
             fp32 matmuls do); ACT exp (scale folded).
  exback(g-1): two PE fp16 transposes + ACT copy bring exp'd dots back to
             token-major ex4.
  sm(g-2):   DVE softmax tail (den/recip/attn-normalize) + numerator
             multiply -> pn [t, h, c, g8].
  out(g-3):  the g-sum folds via 8 parity-accumulating fp32 identity
             matmuls per chunk (stride-8 stationary APs) while transposing
             pn to channel-major; ACT evacuates; PE out-projection
             (contraction 256, wox = Wo^T unexpanded); ACT bias; DMA store.
Engine balance (sim, hot): DVE ~377us, PE ~360us, ACT ~308us, DMA ~141us.
GpSimd is deliberately idle: measured Q7 launch overhead is ~2-5us per op
and heavy Pool traffic degrades concurrent DVE adds (SBUF contention).
DVE TensorTensor runs in 2x mode (fp16, packed innermost dims); all TT APs
must optimize to <= 3 free dims (broadcast (t,h)/(c,g) merges matter).
Custom DVE ops (e.g. reciprocal_approx_fast) crash this runtime
(NRT_EXEC_UNIT_UNRECOVERABLE) - stock ops only.
Measured: 446-447us HW exec (traced, core 0), rel err ~5.4e-4 vs fp32
reference; baseline at session start was 568-570us.
"""

import os
import sys

if "/opt/trn_rl_repo" not in sys.path:
    sys.path.insert(0, "/opt/trn_rl_repo")

from contextlib import ExitStack

import numpy as np

import concourse.bacc as bacc
import concourse.tile as tile
from concourse import mybir
from concourse.bass_utils import run_bass_kernel_spmd
from concourse.masks import make_identity

B, C, HH, WW = 8, 256, 128, 128
NH, HD = 8, 32
N = HH * WW  # tokens per sample
TT = 128  # tokens per sub-tile (partition dim)
G = 4  # sub-tiles per group
GT = G * TT
GR = 1  # residual g extent entering the out-projection (PE folds 8 -> GR)
KCH = (C * GR) // 128  # contraction chunks for the out-projection
SCALE = float(HD) ** -0.5

PRESUM = "pe"  # x-sum rides dual-source PSUM accumulation on PE
GRAM_TILES = 4  # per-sub-tile gram ops (keeps every AP <= 3 free dims)

F32 = mybir.dt.float32
F16 = mybir.dt.float16
AX = mybir.AxisListType


def build_kernel(n_tiles=N // TT, has_qkv_bias=False, has_o_bias=False):
    assert n_tiles % G == 0
    n_groups = n_tiles // G
    nc = bacc.Bacc(trn_type="TRN2")

    xc = nc.declare_dram_parameter("xc", [C, N], F32, isOutput=False)
    xt = nc.declare_dram_parameter("xt", [C, N], F32, isOutput=False)
    wqkv = nc.declare_dram_parameter("wqkv", [C, 3 * C], F16, isOutput=False)
    wox = nc.declare_dram_parameter("wox", [C * GR, C], F16, isOutput=False)
    bqkv = nc.declare_dram_parameter("bqkv", [1, 3 * C], F16, isOutput=False)
    bo = nc.declare_dram_parameter("bo", [C, 1], F32, isOutput=False)
    out = nc.declare_dram_parameter("out", [C, N], F32, isOutput=True)

    with tile.TileContext(nc) as tc, ExitStack() as ctx:
        singles = ctx.enter_context(tc.tile_pool(name="singles", bufs=1))
        xs_pool = ctx.enter_context(tc.tile_pool(name="xs", bufs=2))
        x16_pool = ctx.enter_context(tc.tile_pool(name="x16", bufs=2))
        qkv_pool = ctx.enter_context(tc.tile_pool(name="qkv", bufs=5))
        gp_pool = ctx.enter_context(tc.tile_pool(name="gp", bufs=2))
        pn_pool = ctx.enter_context(tc.tile_pool(name="pn", bufs=3))
        tree_pool = ctx.enter_context(tc.tile_pool(name="tree", bufs=1))
        tree2_pool = ctx.enter_context(tc.tile_pool(name="tree2", bufs=2))
        sm_pool = ctx.enter_context(tc.tile_pool(name="sm", bufs=4))
        otr_pool = ctx.enter_context(tc.tile_pool(name="otr", bufs=2))
        out_pool = ctx.enter_context(tc.tile_pool(name="outp", bufs=2))
        ps_qkv = ctx.enter_context(tc.tile_pool(name="ps_qkv", bufs=2, space="PSUM"))
        ps_tr = ctx.enter_context(tc.tile_pool(name="ps_tr", bufs=2, space="PSUM"))
        ps_out = ctx.enter_context(tc.tile_pool(name="ps_out", bufs=1, space="PSUM"))
        ps_dots = ctx.enter_context(tc.tile_pool(name="ps_dots", bufs=1, space="PSUM"))
        ps_back = ctx.enter_context(tc.tile_pool(name="ps_back", bufs=1, space="PSUM"))

        wqkv_sb = singles.tile([128, 2, 3 * C], F16)
        wox_sb = singles.tile([128, KCH, C], F16)
        ident = singles.tile([128, 128], F16)
        if has_qkv_bias:
            bqkv_sb = singles.tile([1, 3 * C], F16)
            ones_tok = singles.tile([1, TT], F16)
        if has_o_bias:
            bo_sb = singles.tile([128, 2], F32)

        def emit_singles():
            nc.sync.dma_start(
                out=wqkv_sb, in_=wqkv.rearrange("(b p) m -> p b m", b=2)
            )
            nc.sync.dma_start(
                out=wox_sb, in_=wox.rearrange("(k p) m -> p k m", k=KCH)
            )
            make_identity(nc, ident)
            if has_qkv_bias:
                nc.sync.dma_start(out=bqkv_sb, in_=bqkv)
                nc.vector.memset(ones_tok, 1.0)
            if has_o_bias:
                nc.sync.dma_start(
                    out=bo_sb, in_=bo.rearrange("(b p) o -> p (b o)", b=2)
                )

        xc_r = xc.rearrange("(b p) n -> p b n", b=2)
        xt_r = xt.rearrange("(b p) n -> p b n", b=2)
        out_r = out.rearrange("(b p) n -> b p n", b=2)

        st = {}  # per-group state handed between phases

        def emit_load(g):
            # x-sum rides the LOAD itself: SWDGE DMAs cast fp32->fp16 on the
            # fly and the second transfer accumulates into the first
            # (HW-verified: cast+accum DMA is exact)
            gtok = slice(g * GT, (g + 1) * GT)
            x16 = x16_pool.tile([128, 2, GT], F16, tag="x16")
            if g == 0:
                for j in range(G):
                    jt = slice(j * TT, (j + 1) * TT)
                    gjt = slice(g * GT + j * TT, g * GT + (j + 1) * TT)
                    nc.gpsimd.dma_start(out=x16[:, :, jt], in_=xc_r[:, :, gjt])
                    nc.gpsimd.dma_start(
                        out=x16[:, :, jt],
                        in_=xt_r[:, :, gjt],
                        accum_op=mybir.AluOpType.add,
                    )
            else:
                nc.gpsimd.dma_start(out=x16, in_=xc_r[:, :, gtok])
                nc.gpsimd.dma_start(
                    out=x16, in_=xt_r[:, :, gtok], accum_op=mybir.AluOpType.add
                )
            st[g] = {"x16": (x16,), "gtok": gtok}

        def emit_proj(g):
            x_srcs = st[g].pop("x16")
            qkv = qkv_pool.tile([128, G, 3, C], F16, tag="qkv")
            gp = gp_pool.tile([128, G, NH, NH, HD], F16, tag="gp")

            gram_tiles = G if g == 0 else GRAM_TILES
            gstep = G // gram_tiles

            def emit_gram(t0, tn):
                q_b = (
                    qkv[:, t0:tn, 0]
                    .rearrange("p t (h c) -> p t h c", h=NH)
                    .unsqueeze(3)
                    .broadcast_to([TT, tn - t0, NH, NH, HD])
                )
                k_b = (
                    qkv[:, t0:tn, 1]
                    .rearrange("p t (g c) -> p t g c", g=NH)
                    .unsqueeze(2)
                    .broadcast_to([TT, tn - t0, NH, NH, HD])
                )
                nc.vector.tensor_mul(gp[:, t0:tn], q_b, k_b)

            # per sub-tile qkv projection via PSUM accumulation over
            # channel-half b; gram multiply emitted as soon as its input
            # sub-tiles land so the DVE never waits on the full group.
            n_acc = 2 * len(x_srcs) + (1 if has_qkv_bias else 0)
            qkv_flat = qkv.rearrange("p t r m -> p t (r m)")
            for j in range(G):
                for lo, hi in ((0, 384), (384, 768)):
                    qkv_ps = ps_qkv.tile([TT, 384], F32, tag="qkvps")
                    acc = 0
                    for xsrc in x_srcs:
                        for b in range(2):
                            nc.tensor.matmul(
                                qkv_ps,
                                lhsT=xsrc[:, b, j * TT : (j + 1) * TT],
                                rhs=wqkv_sb[:, b, lo:hi],
                                start=(acc == 0),
                                stop=(acc == n_acc - 1),
                            )
                            acc += 1
                    if has_qkv_bias:
                        nc.tensor.matmul(
                            qkv_ps,
                            lhsT=ones_tok,
                            rhs=bqkv_sb[:, lo:hi],
                            start=False,
                            stop=True,
                        )
                    nc.scalar.copy(out=qkv_flat[:, j, lo:hi], in_=qkv_ps)
                if (j + 1) % gstep == 0:
                    emit_gram(j + 1 - gstep, j + 1)

            # gram c-tree: one DVE level (c 32 -> 16); the rest of the
            # reduction rides PE (parity-accumulating matmuls)
            t16 = tree2_pool.tile([128, G * 64, 16], F16, tag="t16")
            tree_slices = (
                [slice(j * 64, (j + 1) * 64) for j in range(G)]
                if g == 0
                else [slice(0, G * 64)]
            )
            gpv = gp.rearrange("p t h g c -> p (t h g) c")  # [128, 256, 32]
            for sl in tree_slices:
                nc.vector.tensor_add(t16[:, sl], gpv[:, sl, 0:16], gpv[:, sl, 16:32])
            st[g]["qkv"] = qkv
            st[g]["t16"] = t16

        def emit_dots(g):
            # c-remnant 8 folded on PE into TRANSPOSED fp32 dots
            # [(rows=(tloc%2,h,g)), chunk=(tloc//2), t]; exp on ACT; two
            # fp16 transposes bring ex back to token-major for the softmax.
            t16 = st[g].pop("t16")
            dots_ps = ps_dots.tile([128, 2, TT], F32, tag="dots")
            for chunk in range(2):
                for par in range(16):
                    nc.tensor.matmul(
                        dots_ps[:, chunk],
                        lhsT=t16[:, chunk * 128 : (chunk + 1) * 128, par],
                        rhs=ident,
                        start=(par == 0),
                        stop=(par == 15),
                    )
            ex_t = sm_pool.tile([128, 2, TT], F16, tag="ex_t")
            nc.scalar.activation(
                out=ex_t.rearrange("p c t -> p (c t)"),
                in_=dots_ps.rearrange("p c t -> p (c t)"),
                func=mybir.ActivationFunctionType.Exp,
                scale=SCALE,
            )
            st[g]["ex_t"] = ex_t

        def emit_exback(g):
            ex_t = st[g].pop("ex_t")
            back_ps = ps_back.tile([128, 2, TT], F16, tag="back")
            for chunk in range(2):
                nc.tensor.transpose(back_ps[:, chunk], ex_t[:, chunk], ident)
            ex4 = sm_pool.tile([128, G, NH, NH], F16, tag="ex4")
            nc.scalar.copy(
                out=ex4.rearrange("p t h g -> p (t h g)"),
                in_=back_ps.rearrange("p c t -> p (c t)"),
            )
            st[g]["ex4"] = ex4

        def emit_sm(g):
            qkv = st[g].pop("qkv")
            ex4 = st[g].pop("ex4")
            # softmax tail (DVE, token-major)
            den4 = sm_pool.tile([128, G * NH], F32, tag="den4")
            nc.vector.reduce_sum(
                den4, ex4.rearrange("p t h g -> p (t h) g"), axis=AX.X
            )
            rec4 = sm_pool.tile([128, G * NH], F16, tag="rec4")
            with nc.allow_low_precision(reason="softmax weights tolerate fp16"):
                nc.vector.reciprocal(rec4, den4)
            attn4 = sm_pool.tile([128, G, NH, NH], F16, tag="attn4")
            nc.vector.tensor_mul(
                attn4.rearrange("p t h g -> p (t h) g"),
                ex4.rearrange("p t h g -> p (t h) g"),
                rec4.unsqueeze(2).broadcast_to([128, G * NH, NH]),
            )

            # numerator multiply: pn [t, h, c, g8] (contiguous layout --
            # all APs merge to <=3 free dims)
            pn = pn_pool.tile([128, G, NH, HD, NH], F16, tag="pn")
            vr = qkv[:, :, 2].rearrange("p t (c g) -> p t c g", g=NH)
            a_b = attn4.unsqueeze(3).broadcast_to([TT, G, NH, HD, NH])
            v_b = vr.unsqueeze(2).broadcast_to([TT, G, NH, HD, NH])
            nc.vector.tensor_mul(pn, a_b, v_b)
            st[g]["pn"] = pn

        def emit_out(g):
            pn = st[g].pop("pn")
            gtok = st[g].pop("gtok")
            del st[g]
            # per sub-tile: the g 8->4 fold rides fp32 PSUM accumulation on
            # PE via REAL matmuls against the identity (fp16 PSUM transposes
            # do not accumulate on TRN2, fp32 matmul PSUM does). Stride-2
            # stationary APs pick even/odd g parities; wox rows repeat WoT
            # per pair so any pairing is valid. Half-tile PSUM chunks keep
            # the PE->ACT evacuation pipelined within the 16KB PSUM budget.
            otr_sb = otr_pool.tile([128, KCH, GT], F16, tag="otrsb")
            for j in range(G):
                pnj = pn[:, j].rearrange("p h c g -> p (h c) g")
                tr_ps = ps_tr.tile([128, KCH, TT], F32, tag="trps")
                for ci in range(KCH):
                    for par in range(NH):
                        nc.tensor.matmul(
                            tr_ps[:, ci],
                            lhsT=pnj[:, ci * 128 : (ci + 1) * 128, par],
                            rhs=ident,
                            start=(par == 0),
                            stop=(par == NH - 1),
                        )
                nc.scalar.copy(
                    out=otr_sb[:, :, j * TT : (j + 1) * TT], in_=tr_ps
                )

            # out-projection with g-expanded Wo': contracts (h,c,g4)=1024
            # in KCH accumulating chunk-matmuls per 128-channel output half.
            out_ps = ps_out.tile([128, 2, GT], F32, tag="outps")
            for m in range(2):
                for ci in range(KCH):
                    nc.tensor.matmul(
                        out_ps[:, m],
                        lhsT=wox_sb[:, ci, m * 128 : (m + 1) * 128],
                        rhs=otr_sb[:, ci],
                        start=(ci == 0),
                        stop=(ci == KCH - 1),
                    )
            out_sb = out_pool.tile([128, 2, GT], F32, tag="outsb")
            if has_o_bias:
                for m in range(2):
                    nc.scalar.activation(
                        out=out_sb[:, m],
                        in_=out_ps[:, m],
                        func=mybir.ActivationFunctionType.Identity,
                        bias=bo_sb[:, m : m + 1],
                    )
            else:
                nc.scalar.copy(out=out_sb, in_=out_ps)
            for b in range(2):
                nc.sync.dma_start(out=out_r[b, :, gtok], in_=out_sb[:, b])

        emitters = {
            "load": emit_load,
            "proj": emit_proj,
            "dots": emit_dots,
            "exback": emit_exback,
            "sm": emit_sm,
            "out": emit_out,
        }
        # emission order + lags tuned so every engine's in-order queue sees
        # ops in dependency-ready order
        schedule = (
            ("load", 0), ("sm", 2), ("proj", 0), ("dots", 1),
            ("exback", 1), ("out", 3),
        )
        emit_singles()
        max_lag = max(lag for _, lag in schedule)
        for i in range(n_groups + max_lag):
            for phase, lag in schedule:
                gi = i - lag
                if 0 <= gi < n_groups:
                    emitters[phase](gi)

    nc.compile()
    return nc


# split_heads uses channel c*nh+h for (head h, dim c); permute projection rows
# so q,k come out head-major (h*32+c). v stays in natural order: its (c,g)
# interleave is exactly what the numerator multiply wants. The out-proj weight
# is expanded over g: pn layout is [h, c, g] so row (h*32+c)*8+g of Wo' equals
# WoT row h*32+c (Wo's input channels already match head-major merge order).
_PERM = np.array([c * NH + h for h in range(NH) for c in range(HD)])

# delta matrices for the PE-side softmax: rows r = (tloc%2)*64 + h*8 + g map
# to den/rec partitions po = (tloc%2)*8 + h
_R = np.arange(128)
_PO = (_R // 64) * 8 + (_R // 8) % 8
DG = np.zeros((128, 16), np.float16)
DG[_R, _PO] = 1.0
DR = np.ascontiguousarray(DG.T)


def _prep_weights(Wq, bq, Wk, bk, Wv, bv, Wo, bo):
    wqkv = np.concatenate([Wq[_PERM].T, Wk[_PERM].T, Wv.T], axis=1)
    wqkv = np.ascontiguousarray(wqkv).astype(np.float16)
    wot = np.ascontiguousarray(Wo.T)  # [256 (h,c), 256 m]
    wox = np.repeat(wot, GR, axis=0).astype(np.float16)  # [(h,c,gGR), 256]
    bqkv = np.concatenate([bq[_PERM], bk[_PERM], bv]).astype(np.float16)[None]
    bo_a = bo.astype(np.float32)[:, None]
    return wqkv, wox, bqkv, bo_a


def _in_maps(x_c, x_t, wqkv, wox, bqkv, bo_a):
    maps = []
    for b in range(B):
        maps.append(
            {
                "xc": np.ascontiguousarray(x_c[b].reshape(C, N)),
                "xt": np.ascontiguousarray(x_t[b].reshape(C, N)),
                "wqkv": wqkv,
                "wox": wox,
                "bqkv": bqkv,
                "bo": bo_a,
            }
        )
    return maps


def kernel(x_c, x_t, Wq, bq, Wk, bk, Wv, bv, Wo, bo):
    x_c = np.asarray(x_c, dtype=np.float32)
    x_t = np.asarray(x_t, dtype=np.float32)
    wqkv, wox, bqkv, bo_a = _prep_weights(
        np.asarray(Wq, np.float32),
        np.asarray(bq, np.float32),
        np.asarray(Wk, np.float32),
        np.asarray(bk, np.float32),
        np.asarray(Wv, np.float32),
        np.asarray(bv, np.float32),
        np.asarray(Wo, np.float32),
        np.asarray(bo, np.float32),
    )
    nc = build_kernel(
        has_qkv_bias=bool(np.any(bqkv)), has_o_bias=bool(np.any(bo_a))
    )
    res = run_bass_kernel_spmd(
        nc, _in_maps(x_c, x_t, wqkv, wox, bqkv, bo_a), list(range(B))
    )
    outs = [np.asarray(res.results[b]["out"]).reshape(C, HH, WW) for b in range(B)]
    return np.stack(outs).astype(np.float32)


def _install_ntff_shim():
    """Recreate the missing antenv.axon_hooks module + ctypes NTFF hook
    (mirrors trn_agent_boot.trn_boot's degraded-silently path). Test-only."""
    import contextlib
    import ctypes
    import types

    try:
        from antenv.axon_hooks import get_axon_ntff_profile_hook  # noqa: F401

        return True
    except ImportError:
        pass
    import antenv

    mod = types.ModuleType("antenv.axon_hooks")
    mod._hook = None

    def set_axon_ntff_profile_hook(h):
        mod._hook = h

    def get_axon_ntff_profile_hook():
        return mod._hook

    mod.set_axon_ntff_profile_hook = set_axon_ntff_profile_hook
    mod.get_axon_ntff_profile_hook = get_axon_ntff_profile_hook
    sys.modules["antenv.axon_hooks"] = mod
    antenv.axon_hooks = mod

    so_path = "/opt/axon/libaxon_pjrt.so"
    if not os.path.exists(so_path):
        return False
    lib = ctypes.CDLL(so_path)
    if not hasattr(lib, "axon_start_nrt_profile"):
        return False
    lib.axon_start_nrt_profile.argtypes = [
        ctypes.POINTER(ctypes.c_int64),
        ctypes.c_size_t,
    ]
    lib.axon_start_nrt_profile.restype = ctypes.c_int64
    lib.axon_stop_nrt_profile.argtypes = [ctypes.c_char_p]
    lib.axon_stop_nrt_profile.restype = ctypes.c_int64

    @contextlib.contextmanager
    def _hook(output_dir, device_ids):
        import jax

        jax.devices()
        if device_ids:
            ids = (ctypes.c_int64 * len(device_ids))(*device_ids)
            rc = lib.axon_start_nrt_profile(ids, len(device_ids))
        else:
            rc = lib.axon_start_nrt_profile(None, 0)
        if rc != 0:
            raise RuntimeError(f"axon_start_nrt_profile rc={rc}")
        try:
            yield
        finally:
            n = lib.axon_stop_nrt_profile(str(output_dir).encode())
            print(f"profile: {n} file(s) written to {output_dir}")

    set_axon_ntff_profile_hook(_hook)
    return True


def profile_run(inputs_np):
    """Run once more with NTFF tracing on core 0; return exec_time_ns."""
    import concourse.bass_utils as bu

    _install_ntff_shim()
    bu.upload_artifacts = lambda d: "local://" + d  # no S3 in this container
    x_c = np.asarray(inputs_np["x_c"], np.float32)
    x_t = np.asarray(inputs_np["x_t"], np.float32)
    wqkv, wox, bqkv, bo_a = _prep_weights(
        *[
            np.asarray(inputs_np[k], np.float32)
            for k in ("Wq", "bq", "Wk", "bk", "Wv", "bv", "Wo", "bo")
        ]
    )
    nc = build_kernel(
        has_qkv_bias=bool(np.any(bqkv)), has_o_bias=bool(np.any(bo_a))
    )
    res = run_bass_kernel_spmd(
        nc,
        _in_maps(x_c, x_t, wqkv, wox, bqkv, bo_a),
        list(range(B)),
        trace=True,
    )
    return res.exec_time_ns


if __name__ == "__main__":
    rng = np.random.default_rng(0)
    ins = {
        "x_c": rng.standard_normal((B, C, HH, WW), dtype=np.float32),
        "x_t": rng.standard_normal((B, C, HH, WW), dtype=np.float32),
        "Wq": (rng.standard_normal((C, C)) * 0.02).astype(np.float32),
        "bq": np.zeros(C, np.float32),
        "Wk": (rng.standard_normal((C, C)) * 0.02).astype(np.float32),
        "bk": np.zeros(C, np.float32),
        "Wv": (rng.standard_normal((C, C)) * 0.02).astype(np.float32),
        "bv": np.zeros(C, np.float32),
        "Wo": (rng.standard_normal((C, C)) * 0.02).astype(np.float32),
        "bo": np.zeros(C, np.float32),
    }
    out = kernel(**ins)
    print(out.shape, out.dtype)


# revision 70
# speedup vs baseline: 1.1888x; 1.0536x over previous
"""Trainium2 Bass kernel: fused cross-head attention block (data parallel, 8 cores).

Problem (hardcoded shapes): x_c, x_t [8, 256, 128, 128] f32; Wq/Wk/Wv/Wo
[256, 256]; biases [256]; out [8, 256, 128, 128] f32.

Math per sample (C=256, nh=8, hd=32, N=H*W=16384 tokens):
  x = x_c + x_t;  q/k/v = per-token linear projections of x
  per token: dot[h,g] = q_h . k_g * hd^-0.5   (8x8 gram ACROSS heads)
             attn = softmax_g(dot);  o_h = sum_g attn[h,g] v_g
  out = Wo @ o (+ bo)

Sharding: pure data parallel - one sample per NeuronCore, weights replicated.

Per core: 32 groups of 4x128-token tiles, software-pipelined in 6 phases
(lags in groups) so every in-order engine queue sees work dependency-ready:
  load(g):   DMA xc/xt; fp16 casts on ACT (the x-sum rides PE's PSUM
             accumulation over both sources in the qkv matmuls).
  proj(g):   PE qkv matmuls (384-wide PSUM half-tiles, dual-source + c-half
             accumulated); ACT evacuation; DVE per-tile gram multiply
             (broadcast APs) + ONE c-tree level (32 -> 16).
  dots(g-1): the remaining c-reduction rides PE: 16 parity-accumulating
             identity matmuls per 128-row chunk land TRANSPOSED fp32 dots
             in PSUM (fp16 PSUM transposes do NOT accumulate on TRN2; real
             fp32 matmuls do); ACT exp (scale folded).
  exback(g-1): two PE fp16 transposes + ACT copy bring exp'd dots back to
             token-major ex4.
  sm(g-2):   DVE softmax tail (den/recip/attn-normalize) + numerator
             multiply -> pn [t, h, c, g8].
  out(g-3):  the g-sum folds via 8 parity-accumulating fp32 identity
             matmuls per chunk (stride-8 stationary APs) while transposing
             pn to channel-major; ACT evacuates; PE out-projection
             (contraction 256, wox = Wo^T unexpanded); ACT bias; DMA store.
Engine balance (sim, hot): DVE ~377us, PE ~360us, ACT ~308us, DMA ~141us.
GpSimd is deliberately idle: measured Q7 launch overhead is ~2-5us per op
and heavy Pool traffic degrades concurrent DVE adds (SBUF contention).
DVE TensorTensor runs in 2x mode (fp16, packed innermost dims); all TT APs
must optimize to <= 3 free dims (broadcast (t,h)/(c,g) merges matter).
Custom DVE ops (e.g. reciprocal_approx_fast) crash this runtime
(NRT_EXEC_UNIT_UNRECOVERABLE) - stock ops only.
Measured: 446-447us HW exec (traced, core 0), rel err ~5.4e-4 vs fp32
reference; baseline at session start was 568-570us.
"""

import os
import sys

if "/opt/trn_rl_repo" not in sys.path:
    sys.path.insert(0, "/opt/trn_rl_repo")

from contextlib import ExitStack

import numpy as np

import concourse.bacc as bacc
import concourse.tile as tile
from concourse import mybir
from concourse.bass_utils import run_bass_kernel_spmd
from concourse.masks import make_identity

B, C, HH, WW = 8, 256, 128, 128
NH, HD = 8, 32
N = HH * WW  # tokens per sample
TT = 128  # tokens per sub-tile (partition dim)
G = 4  # sub-tiles per group
GT = G * TT
GR = 1  # residual g extent entering the out-projection (PE folds 8 -> GR)
KCH = (C * GR) // 128  # contraction chunks for the out-projection
SCALE = float(HD) ** -0.5

PRESUM = "pe"  # x-sum rides dual-source PSUM accumulation on PE
GRAM_TILES = 4  # per-sub-tile gram ops (keeps every AP <= 3 free dims)

F32 = mybir.dt.float32
F16 = mybir.dt.float16
AX = mybir.AxisListType


def build_kernel(n_tiles=N // TT, has_qkv_bias=False, has_o_bias=False):
    assert n_tiles % G == 0
    n_groups = n_tiles // G
    nc = bacc.Bacc(trn_type="TRN2")

    xc = nc.declare_dram_parameter("xc", [C, N], F32, isOutput=False)
    xt = nc.declare_dram_parameter("xt", [C, N], F32, isOutput=False)
    wqkv = nc.declare_dram_parameter("wqkv", [C, 3 * C], F16, isOutput=False)
    wox = nc.declare_dram_parameter("wox", [C * GR, C], F16, isOutput=False)
    bqkv = nc.declare_dram_parameter("bqkv", [1, 3 * C], F16, isOutput=False)
    bo = nc.declare_dram_parameter("bo", [C, 1], F32, isOutput=False)
    out = nc.declare_dram_parameter("out", [C, N], F32, isOutput=True)

    with tile.TileContext(nc) as tc, ExitStack() as ctx:
        singles = ctx.enter_context(tc.tile_pool(name="singles", bufs=1))
        xs_pool = ctx.enter_context(tc.tile_pool(name="xs", bufs=2))
        x16_pool = ctx.enter_context(tc.tile_pool(name="x16", bufs=2))
        qkv_pool = ctx.enter_context(tc.tile_pool(name="qkv", bufs=5))
        gp_pool = ctx.enter_context(tc.tile_pool(name="gp", bufs=2))
        pn_pool = ctx.enter_context(tc.tile_pool(name="pn", bufs=3))
        tree_pool = ctx.enter_context(tc.tile_pool(name="tree", bufs=1))
        tree2_pool = ctx.enter_context(tc.tile_pool(name="tree2", bufs=2))
        sm_pool = ctx.enter_context(tc.tile_pool(name="sm", bufs=4))
        otr_pool = ctx.enter_context(tc.tile_pool(name="otr", bufs=2))
        out_pool = ctx.enter_context(tc.tile_pool(name="outp", bufs=2))
        ps_qkv = ctx.enter_context(tc.tile_pool(name="ps_qkv", bufs=2, space="PSUM"))
        ps_tr = ctx.enter_context(tc.tile_pool(name="ps_tr", bufs=2, space="PSUM"))
        ps_out = ctx.enter_context(tc.tile_pool(name="ps_out", bufs=1, space="PSUM"))
        ps_dots = ctx.enter_context(tc.tile_pool(name="ps_dots", bufs=1, space="PSUM"))
        ps_back = ctx.enter_context(tc.tile_pool(name="ps_back", bufs=1, space="PSUM"))

        wqkv_sb = singles.tile([128, 2, 3 * C], F16)
        wox_sb = singles.tile([128, KCH, C], F16)
        ident = singles.tile([128, 128], F16)
        if has_qkv_bias:
            bqkv_sb = singles.tile([1, 3 * C], F16)
            ones_tok = singles.tile([1, TT], F16)
        if has_o_bias:
            bo_sb = singles.tile([128, 2], F32)

        def emit_singles():
            nc.sync.dma_start(
                out=wqkv_sb, in_=wqkv.rearrange("(b p) m -> p b m", b=2)
            )
            nc.sync.dma_start(
                out=wox_sb, in_=wox.rearrange("(k p) m -> p k m", k=KCH)
            )
            make_identity(nc, ident)
            if has_qkv_bias:
                nc.sync.dma_start(out=bqkv_sb, in_=bqkv)
                nc.vector.memset(ones_tok, 1.0)
            if has_o_bias:
                nc.sync.dma_start(
                    out=bo_sb, in_=bo.rearrange("(b p) o -> p (b o)", b=2)
                )

        xc_r = xc.rearrange("(b p) n -> p b n", b=2)
        xt_r = xt.rearrange("(b p) n -> p b n", b=2)
        out_r = out.rearrange("(b p) n -> b p n", b=2)

        st = {}  # per-group state handed between phases

        def emit_load(g):
            # x-sum rides the LOAD itself: SWDGE DMAs cast fp32->fp16 on the
            # fly and the second transfer accumulates into the first
            # (HW-verified: cast+accum DMA is exact)
            gtok = slice(g * GT, (g + 1) * GT)
            x16 = x16_pool.tile([128, 2, GT], F16, tag="x16")
            if g == 0:
                for j in range(G):
                    jt = slice(j * TT, (j + 1) * TT)
                    gjt = slice(g * GT + j * TT, g * GT + (j + 1) * TT)
                    nc.gpsimd.dma_start(out=x16[:, :, jt], in_=xc_r[:, :, gjt])
                    nc.gpsimd.dma_start(
                        out=x16[:, :, jt],
                        in_=xt_r[:, :, gjt],
                        accum_op=mybir.AluOpType.add,
                    )
            else:
                nc.gpsimd.dma_start(out=x16, in_=xc_r[:, :, gtok])
                nc.gpsimd.dma_start(
                    out=x16, in_=xt_r[:, :, gtok], accum_op=mybir.AluOpType.add
                )
            st[g] = {"x16": (x16,), "gtok": gtok}

        def emit_proj(g):
            x_srcs = st[g].pop("x16")
            qkv = qkv_pool.tile([128, G, 3, C], F16, tag="qkv")
            gp = gp_pool.tile([128, G, NH, NH, HD], F16, tag="gp")

            gram_tiles = G if g == 0 else GRAM_TILES
            gstep = G // gram_tiles

            def emit_gram(t0, tn):
                q_b = (
                    qkv[:, t0:tn, 0]
                    .rearrange("p t (h c) -> p t h c", h=NH)
                    .unsqueeze(3)
                    .broadcast_to([TT, tn - t0, NH, NH, HD])
                )
                k_b = (
                    qkv[:, t0:tn, 1]
                    .rearrange("p t (g c) -> p t g c", g=NH)
                    .unsqueeze(2)
                    .broadcast_to([TT, tn - t0, NH, NH, HD])
                )
                nc.vector.tensor_mul(gp[:, t0:tn], q_b, k_b)

            # per sub-tile qkv projection via PSUM accumulation over
            # channel-half b; gram multiply emitted as soon as its input
            # sub-tiles land so the DVE never waits on the full group.
            n_acc = 2 * len(x_srcs) + (1 if has_qkv_bias else 0)
            qkv_flat = qkv.rearrange("p t r m -> p t (r m)")
            for j in range(G):
                for lo, hi in ((0, 384), (384, 768)):
                    qkv_ps = ps_qkv.tile([TT, 384], F32, tag="qkvps")
                    acc = 0
                    for xsrc in x_srcs:
                        for b in range(2):
                            nc.tensor.matmul(
                                qkv_ps,
                                lhsT=xsrc[:, b, j * TT : (j + 1) * TT],
                                rhs=wqkv_sb[:, b, lo:hi],
                                start=(acc == 0),
                                stop=(acc == n_acc - 1),
                            )
                            acc += 1
                    if has_qkv_bias:
                        nc.tensor.matmul(
                            qkv_ps,
                            lhsT=ones_tok,
                            rhs=bqkv_sb[:, lo:hi],
                            start=False,
                            stop=True,
                        )
                    nc.scalar.copy(out=qkv_flat[:, j, lo:hi], in_=qkv_ps)
                if (j + 1) % gstep == 0:
                    emit_gram(j + 1 - gstep, j + 1)

            # the whole gram c-reduction rides PE (32 parity-accumulating
            # matmuls per 128-row chunk, fp32 PSUM)
            st[g]["qkv"] = qkv
            st[g]["gp"] = gp

        def emit_dots(g):
            # c-remnant 8 folded on PE into TRANSPOSED fp32 dots
            # [(rows=(tloc%2,h,g)), chunk=(tloc//2), t]; exp on ACT; two
            # fp16 transposes bring ex back to token-major for the softmax.
            gp = st[g].pop("gp")
            gpv = gp.rearrange("p t h g c -> p (t h g) c")  # [128, 256, 32]
            dots_ps = ps_dots.tile([128, 2, TT], F32, tag="dots")
            for chunk in range(2):
                for par in range(32):
                    nc.tensor.matmul(
                        dots_ps[:, chunk],
                        lhsT=gpv[:, chunk * 128 : (chunk + 1) * 128, par],
                        rhs=ident,
                        start=(par == 0),
                        stop=(par == 31),
                    )
            ex_t = sm_pool.tile([128, 2, TT], F16, tag="ex_t")
            nc.scalar.activation(
                out=ex_t.rearrange("p c t -> p (c t)"),
                in_=dots_ps.rearrange("p c t -> p (c t)"),
                func=mybir.ActivationFunctionType.Exp,
                scale=SCALE,
            )
            st[g]["ex_t"] = ex_t

        def emit_exback(g):
            ex_t = st[g].pop("ex_t")
            back_ps = ps_back.tile([128, 2, TT], F16, tag="back")
            for chunk in range(2):
                nc.tensor.transpose(back_ps[:, chunk], ex_t[:, chunk], ident)
            ex4 = sm_pool.tile([128, G, NH, NH], F16, tag="ex4")
            nc.scalar.copy(
                out=ex4.rearrange("p t h g -> p (t h g)"),
                in_=back_ps.rearrange("p c t -> p (c t)"),
            )
            st[g]["ex4"] = ex4

        def emit_sm(g):
            qkv = st[g].pop("qkv")
            ex4 = st[g].pop("ex4")
            # softmax tail (DVE, token-major)
            den4 = sm_pool.tile([128, G * NH], F32, tag="den4")
            nc.vector.reduce_sum(
                den4, ex4.rearrange("p t h g -> p (t h) g"), axis=AX.X
            )
            rec4 = sm_pool.tile([128, G * NH], F16, tag="rec4")
            with nc.allow_low_precision(reason="softmax weights tolerate fp16"):
                nc.vector.reciprocal(rec4, den4)
            attn4 = sm_pool.tile([128, G, NH, NH], F16, tag="attn4")
            nc.vector.tensor_mul(
                attn4.rearrange("p t h g -> p (t h) g"),
                ex4.rearrange("p t h g -> p (t h) g"),
                rec4.unsqueeze(2).broadcast_to([128, G * NH, NH]),
            )

            # numerator multiply: pn [t, h, c, g8] (contiguous layout --
            # all APs merge to <=3 free dims)
            pn = pn_pool.tile([128, G, NH, HD, NH], F16, tag="pn")
            vr = qkv[:, :, 2].rearrange("p t (c g) -> p t c g", g=NH)
            a_b = attn4.unsqueeze(3).broadcast_to([TT, G, NH, HD, NH])
            v_b = vr.unsqueeze(2).broadcast_to([TT, G, NH, HD, NH])
            nc.vector.tensor_mul(pn, a_b, v_b)
            st[g]["pn"] = pn

        def emit_out(g):
            pn = st[g].pop("pn")
            gtok = st[g].pop("gtok")
            del st[g]
            # per sub-tile: the g 8->4 fold rides fp32 PSUM accumulation on
            # PE via REAL matmuls against the identity (fp16 PSUM transposes
            # do not accumulate on TRN2, fp32 matmul PSUM does). Stride-2
            # stationary APs pick even/odd g parities; wox rows repeat WoT
            # per pair so any pairing is valid. Half-tile PSUM chunks keep
            # the PE->ACT evacuation pipelined within the 16KB PSUM budget.
            otr_sb = otr_pool.tile([128, KCH, GT], F16, tag="otrsb")
            for j in range(G):
                pnj = pn[:, j].rearrange("p h c g -> p (h c) g")
                tr_ps = ps_tr.tile([128, KCH, TT], F32, tag="trps")
                for ci in range(KCH):
                    for par in range(NH):
                        nc.tensor.matmul(
                            tr_ps[:, ci],
                            lhsT=pnj[:, ci * 128 : (ci + 1) * 128, par],
                            rhs=ident,
                            start=(par == 0),
                            stop=(par == NH - 1),
                        )
                nc.scalar.copy(
                    out=otr_sb[:, :, j * TT : (j + 1) * TT], in_=tr_ps
                )

            # out-projection with g-expanded Wo': contracts (h,c,g4)=1024
            # in KCH accumulating chunk-matmuls per 128-channel output half.
            out_ps = ps_out.tile([128, 2, GT], F32, tag="outps")
            for m in range(2):
                for ci in range(KCH):
                    nc.tensor.matmul(
                        out_ps[:, m],
                        lhsT=wox_sb[:, ci, m * 128 : (m + 1) * 128],
                        rhs=otr_sb[:, ci],
                        start=(ci == 0),
                        stop=(ci == KCH - 1),
                    )
            out_sb = out_pool.tile([128, 2, GT], F32, tag="outsb")
            if has_o_bias:
                for m in range(2):
                    nc.scalar.activation(
                        out=out_sb[:, m],
                        in_=out_ps[:, m],
                        func=mybir.ActivationFunctionType.Identity,
                        bias=bo_sb[:, m : m + 1],
                    )
            else:
                nc.scalar.copy(out=out_sb, in_=out_ps)
            for b in range(2):
                nc.sync.dma_start(out=out_r[b, :, gtok], in_=out_sb[:, b])

        emitters = {
            "load": emit_load,
            "proj": emit_proj,
            "dots": emit_dots,
            "exback": emit_exback,
            "sm": emit_sm,
            "out": emit_out,
        }
        # emission order + lags tuned so every engine's in-order queue sees
        # ops in dependency-ready order
        schedule = (
            ("load", 0), ("sm", 2), ("proj", 0), ("dots", 1),
            ("exback", 1), ("out", 3),
        )
        emit_singles()
        max_lag = max(lag for _, lag in schedule)
        for i in range(n_groups + max_lag):
            for phase, lag in schedule:
                gi = i - lag
                if 0 <= gi < n_groups:
                    emitters[phase](gi)

    nc.compile()
    return nc


# split_heads uses channel c*nh+h for (head h, dim c); permute projection rows
# so q,k come out head-major (h*32+c). v stays in natural order: its (c,g)
# interleave is exactly what the numerator multiply wants. The out-proj weight
# is expanded over g: pn layout is [h, c, g] so row (h*32+c)*8+g of Wo' equals
# WoT row h*32+c (Wo's input channels already match head-major merge order).
_PERM = np.array([c * NH + h for h in range(NH) for c in range(HD)])

# delta matrices for the PE-side softmax: rows r = (tloc%2)*64 + h*8 + g map
# to den/rec partitions po = (tloc%2)*8 + h
_R = np.arange(128)
_PO = (_R // 64) * 8 + (_R // 8) % 8
DG = np.zeros((128, 16), np.float16)
DG[_R, _PO] = 1.0
DR = np.ascontiguousarray(DG.T)


def _prep_weights(Wq, bq, Wk, bk, Wv, bv, Wo, bo):
    wqkv = np.concatenate([Wq[_PERM].T, Wk[_PERM].T, Wv.T], axis=1)
    wqkv = np.ascontiguousarray(wqkv).astype(np.float16)
    wot = np.ascontiguousarray(Wo.T)  # [256 (h,c), 256 m]
    wox = np.repeat(wot, GR, axis=0).astype(np.float16)  # [(h,c,gGR), 256]
    bqkv = np.concatenate([bq[_PERM], bk[_PERM], bv]).astype(np.float16)[None]
    bo_a = bo.astype(np.float32)[:, None]
    return wqkv, wox, bqkv, bo_a


def _in_maps(x_c, x_t, wqkv, wox, bqkv, bo_a):
    maps = []
    for b in range(B):
        maps.append(
            {
                "xc": np.ascontiguousarray(x_c[b].reshape(C, N)),
                "xt": np.ascontiguousarray(x_t[b].reshape(C, N)),
                "wqkv": wqkv,
                "wox": wox,
                "bqkv": bqkv,
                "bo": bo_a,
            }
        )
    return maps


def kernel(x_c, x_t, Wq, bq, Wk, bk, Wv, bv, Wo, bo):
    x_c = np.asarray(x_c, dtype=np.float32)
    x_t = np.asarray(x_t, dtype=np.float32)
    wqkv, wox, bqkv, bo_a = _prep_weights(
        np.asarray(Wq, np.float32),
        np.asarray(bq, np.float32),
        np.asarray(Wk, np.float32),
        np.asarray(bk, np.float32),
        np.asarray(Wv, np.float32),
        np.asarray(bv, np.float32),
        np.asarray(Wo, np.float32),
        np.asarray(bo, np.float32),
    )
    nc = build_kernel(
        has_qkv_bias=bool(np.any(bqkv)), has_o_bias=bool(np.any(bo_a))
    )
    res = run_bass_kernel_spmd(
        nc, _in_maps(x_c, x_t, wqkv, wox, bqkv, bo_a), list(range(B))
    )
    outs = [np.asarray(res.results[b]["out"]).reshape(C, HH, WW) for b in range(B)]
    return np.stack(outs).astype(np.float32)


def _install_ntff_shim():
    """Recreate the missing antenv.axon_hooks module + ctypes NTFF hook
    (mirrors trn_agent_boot.trn_boot's degraded-silently path). Test-only."""
    import contextlib
    import ctypes
    import types

    try:
        from antenv.axon_hooks import get_axon_ntff_profile_hook  # noqa: F401

        return True
    except ImportError:
        pass
    import antenv

    mod = types.ModuleType("antenv.axon_hooks")
    mod._hook = None

    def set_axon_ntff_profile_hook(h):
        mod._hook = h

    def get_axon_ntff_profile_hook():
        return mod._hook

    mod.set_axon_ntff_profile_hook = set_axon_ntff_profile_hook
    mod.get_axon_ntff_profile_hook = get_axon_ntff_profile_hook
    sys.modules["antenv.axon_hooks"] = mod
    antenv.axon_hooks = mod

    so_path = "/opt/axon/libaxon_pjrt.so"
    if not os.path.exists(so_path):
        return False
    lib = ctypes.CDLL(so_path)
    if not hasattr(lib, "axon_start_nrt_profile"):
        return False
    lib.axon_start_nrt_profile.argtypes = [
        ctypes.POINTER(ctypes.c_int64),
        ctypes.c_size_t,
    ]
    lib.axon_start_nrt_profile.restype = ctypes.c_int64
    lib.axon_stop_nrt_profile.argtypes = [ctypes.c_char_p]
    lib.axon_stop_nrt_profile.restype = ctypes.c_int64

    @contextlib.contextmanager
    def _hook(output_dir, device_ids):
        import jax

        jax.devices()
        if device_ids:
            ids = (ctypes.c_int64 * len(device_ids))(*device_ids)
            rc = lib.axon_start_nrt_profile(ids, len(device_ids))
        else:
            rc = lib.axon_start_nrt_profile(None, 0)
        if rc != 0:
            raise RuntimeError(f"axon_start_nrt_profile rc={rc}")
        try:
            yield
        finally:
            n = lib.axon_stop_nrt_profile(str(output_dir).encode())
            print(f"profile: {n} file(s) written to {output_dir}")

    set_axon_ntff_profile_hook(_hook)
    return True


def profile_run(inputs_np):
    """Run once more with NTFF tracing on core 0; return exec_time_ns."""
    import concourse.bass_utils as bu

    _install_ntff_shim()
    bu.upload_artifacts = lambda d: "local://" + d  # no S3 in this container
    x_c = np.asarray(inputs_np["x_c"], np.float32)
    x_t = np.asarray(inputs_np["x_t"], np.float32)
    wqkv, wox, bqkv, bo_a = _prep_weights(
        *[
            np.asarray(inputs_np[k], np.float32)
            for k in ("Wq", "bq", "Wk", "bk", "Wv", "bv", "Wo", "bo")
        ]
    )
    nc = build_kernel(
        has_qkv_bias=bool(np.any(bqkv)), has_o_bias=bool(np.any(bo_a))
    )
    res = run_bass_kernel_spmd(
        nc,
        _in_maps(x_c, x_t, wqkv, wox, bqkv, bo_a),
        list(range(B)),
        trace=True,
    )
    return res.exec_time_ns


if __name__ == "__main__":
    rng = np.random.default_rng(0)
    ins = {
        "x_c": rng.standard_normal((B, C, HH, WW), dtype=np.float32),
        "x_t": rng.standard_normal((B, C, HH, WW), dtype=np.float32),
        "Wq": (rng.standard_normal((C, C)) * 0.02).astype(np.float32),
        "bq": np.zeros(C, np.float32),
        "Wk": (rng.standard_normal((C, C)) * 0.02).astype(np.float32),
        "bk": np.zeros(C, np.float32),
        "Wv": (rng.standard_normal((C, C)) * 0.02).astype(np.float32),
        "bv": np.zeros(C, np.float32),
        "Wo": (rng.standard_normal((C, C)) * 0.02).astype(np.float32),
        "bo": np.zeros(C, np.float32),
    }
    out = kernel(**ins)
    print(out.shape, out.dtype)
